# revision 8
# baseline (speedup 1.0000x reference)
"""Trainium2 Bass kernel for the fused broadcast multiply-add:

    out[s, i, f] = x[s, i] * W[i, f] + b[i, f]

Shapes (hardcoded): x [16384, 32] f32, W [32, 256] f32, b [32, 256] f32,
out [16384, 32, 256] f32 (512 MB) -- a pure HBM-write-bound problem.

Strategy
--------
Data parallel over 8 NeuronCores: each core handles 2048 batch rows and
writes a 64 MB output shard. The kernel is output-DMA-bound; everything
else is engineered to keep the 16 SDMA engines at line rate end to end.

Compute: single-fp16 TensorE matmuls (the 2e-2 relative-error budget is
~20x looser than a single fp16 product, so no hi/lo split). Each 512-col
chunk n covers i = {2n, 2n+1} and contracts over K=4 rows at partition
base 32*(n%4) (LDWEIGHTS requires 32-aligned bases), slot s=n//4 on the
free dim:

    lhsT rows: x[i0], x[i1], 1, 0       rhs rows: W[i0]|0, 0|W[i1], b, 0

Inputs load as compact per-group DMAs (0.26 MB total vs 2.5 MB for the
hi/lo baseline), so the store stream is essentially output bytes only.

Store-stripe rotation: traces show that on ~3 of 8 cores exactly one
SDMA engine runs ~15% slower for the whole kernel (HBM address-stripe
contention; which engine varies by core), and with the natural store AP
every engine serves a FIXED output-row stripe for the entire kernel, so
a slow stripe pins to one engine -- a ~22 us serial tail while the
other 15 idle. Fix: each store keeps the SBUF walk natural (full 128
partitions in one evenly-dealt transfer; partition-partial transfers
deal descriptors 7:1 onto the low engines -- measured) but factorizes
the DRAM-side row dim as (a b) c -> b a c with a per-tile split, which
pairs SBUF partition o with DRAM row perm(o) = (o%a)*(128/a) + o//a in
byte-stream order (verified on HW). The host permutes the batch columns
of the activation tile to compensate, so data lands correctly while
every engine's HBM stripe cycles through 7 distinct patterns and a slow
stripe is time-shared instead of pinned.
"""

import numpy as np

import concourse.bass as bass
import concourse.bacc as bacc
import concourse.mybir as mybir
import concourse.tile as tile
from concourse import bass_utils

BS, DEMO, FEAT = 16384, 32, 256
NCORES = 8
BSH = BS // NCORES        # 2048 batch rows per core
PT = 128                  # batch rows per matmul tile (out partitions)
NTILES = BSH // PT        # 16
NF = DEMO * FEAT          # 8192 output columns
NCHUNK = 512              # fp32 columns per PSUM bank / matmul
NCH = NF // NCHUNK        # 16 chunks (each covers two i values)
NSLOT = NCH // 4          # 4 free-dim slots per row-group
KR = 4                    # lhsT rows per chunk: x[i0], x[i1], ones, zeros

# DRAM-row-dim factorizations cycled per tile: None = identity,
# a = outer split of "(a b) c -> b a c". Each yields a distinct
# engine->row-stripe permutation while keeping all 128 partitions in
# one evenly-dealt transfer.
FACT = (None, 16, 8, 32, 4, 64, 2)


def _perm(fa):
    """Row permutation induced by the dest-side (a b) -> b a rearrange:
    SBUF partition o lands on DRAM row _perm(fa)[o]."""
    o = np.arange(PT)
    if fa is None:
        return o
    return (o % fa) * (PT // fa) + o // fa

_cache: dict = {}


def _build():
    nc = bacc.Bacc("TRN2", target_bir_lowering=False, debug=False)

    # Compact inputs: row 4r+k of xap_d/wbp_d is row k of group r.
    xap_d = nc.dram_tensor(
        "xap", (4 * KR, NSLOT * BSH), mybir.dt.float16, kind="ExternalInput"
    )
    wbp_d = nc.dram_tensor(
        "wbp", (4 * KR, NSLOT * NCHUNK), mybir.dt.float16, kind="ExternalInput"
    )
    out_d = nc.dram_tensor("out", (BSH, NF), mybir.dt.float32, kind="ExternalOutput")

    with tile.TileContext(nc) as tc:
        with (
            tc.tile_pool(name="const", bufs=1) as cpool,
            tc.tile_pool(name="opool", bufs=4) as opool,
            tc.tile_pool(name="psum", bufs=4, space=bass.MemorySpace.PSUM) as psum,
        ):
            wbp_t = cpool.tile([128, NSLOT * NCHUNK], mybir.dt.float16)
            xap_t = cpool.tile([128, NSLOT * BSH], mybir.dt.float16)
            for r in range(4):
                nc.sync.dma_start(
                    wbp_t[32 * r:32 * r + KR, :], wbp_d.ap()[KR * r:KR * (r + 1), :]
                )
                nc.sync.dma_start(
                    xap_t[32 * r:32 * r + KR, :], xap_d.ap()[KR * r:KR * (r + 1), :]
                )

            for t in range(NTILES):
                o_t = opool.tile([PT, NF], mybir.dt.float32)
                fa = FACT[t % len(FACT)]
                for g in range(8):  # copy groups of 1024 cols (2 chunks)
                    acc = psum.tile([PT, 2 * NCHUNK], mybir.dt.float32)
                    for h in range(2):
                        n = 2 * g + h
                        r, s = n % 4, n // 4
                        nc.tensor.matmul(
                            acc[:, h * NCHUNK:(h + 1) * NCHUNK],
                            xap_t[32 * r:32 * r + KR,
                                  s * BSH + t * PT: s * BSH + (t + 1) * PT],
                            wbp_t[32 * r:32 * r + KR,
                                  s * NCHUNK:(s + 1) * NCHUNK],
                            start=True,
                            stop=True,
                            tile_position=(32 * r, 0),
                        )
                    dst = o_t[:, g * 1024:(g + 1) * 1024]
                    if g % 2 == 0:
                        nc.vector.tensor_copy(dst, acc[:])
                    else:
                        nc.scalar.copy(dst, acc[:])
                    if g % 2 == 1:  # 1 MB quarter-tile stores
                        lo, hi = (g - 1) * 1024, (g + 1) * 1024
                        dest = out_d.ap()[t * PT:(t + 1) * PT, lo:hi]
                        if fa is not None:
                            # dest-side only: partition o -> row perm(o)
                            dest = dest.rearrange("(a b) c -> b a c", a=fa)
                        nc.sync.dma_start(dest, o_t[:, lo:hi])

    nc.compile()
    return nc


def _get_nc():
    if "nc" not in _cache:
        _cache["nc"] = _build()
    return _cache["nc"]


def _prep_wbp(W, b):
    W16 = np.asarray(W, dtype=np.float32).astype(np.float16)
    b16 = np.asarray(b, dtype=np.float32).astype(np.float16)
    wbp = np.zeros((4 * KR, NSLOT * NCHUNK), dtype=np.float16)
    for n in range(NCH):
        r, s = n % 4, n // 4
        p0, c0 = KR * r, s * NCHUNK
        wbp[p0 + 0, c0:c0 + FEAT] = W16[2 * n]
        wbp[p0 + 1, c0 + FEAT:c0 + 2 * FEAT] = W16[2 * n + 1]
        wbp[p0 + 2, c0:c0 + FEAT] = b16[2 * n]
        wbp[p0 + 2, c0 + FEAT:c0 + 2 * FEAT] = b16[2 * n + 1]
    return wbp


def _prep_xap(x_shard):
    """[BSH, DEMO] f32 -> [16, NSLOT*BSH] fp16 lhsT layout, with batch
    columns permuted per tile to compensate the store-side row perm."""
    x16 = np.asarray(x_shard, dtype=np.float32).astype(np.float16)
    xp = np.empty_like(x16)
    for t in range(NTILES):
        p = _perm(FACT[t % len(FACT)])
        xp[t * PT:(t + 1) * PT] = x16[t * PT + p]
    xT = np.ascontiguousarray(xp.T)  # [DEMO, BSH]
    xap = np.zeros((4 * KR, NSLOT * BSH), dtype=np.float16)
    for n in range(NCH):
        r, s = n % 4, n // 4
        p0 = KR * r
        xs = slice(s * BSH, (s + 1) * BSH)
        xap[p0 + 0, xs] = xT[2 * n]
        xap[p0 + 1, xs] = xT[2 * n + 1]
        xap[p0 + 2, xs] = 1.0
    return xap


def _in_maps(x, W, b):
    wbp = _prep_wbp(W, b)
    x = np.asarray(x, dtype=np.float32)
    return [
        {"xap": _prep_xap(x[c * BSH:(c + 1) * BSH]), "wbp": wbp}
        for c in range(NCORES)
    ]


def run_shards(x, W, b, **spmd_kwargs):
    """Run the SPMD kernel; returns the BassKernelResults (for profiling)."""
    nc = _get_nc()
    return bass_utils.run_bass_kernel_spmd(
        nc, _in_maps(x, W, b), core_ids=list(range(NCORES)), **spmd_kwargs
    )


def kernel(x, W, b):
    res = run_shards(x, W, b)
    out = np.concatenate([res.results[c]["out"] for c in range(NCORES)], axis=0)
    return out.reshape(BS, DEMO, FEAT)
